# revision 1
# baseline (speedup 1.0000x reference)
"""Multi-head self-attention (dense transformer block) on 8 Trainium2 cores.

Tensor-parallel over heads: core m handles heads {2m, 2m+1} for both batch
elements. The reference's RoPE uses angles that depend only on the head
index (not the position), so it is a fixed orthogonal rotation per head;
we fold it (and the 1/sqrt(D) score scale) into the QKV weights on the
host. Device pipeline per core:

  1. qT/kT/vT = (w_slice)^T @ x^T          [d-major, tokens on free dim]
  2. V~ = transpose(vT) with a ones column appended per head
  3. per (batch, head): ST = k^T q  -> exp -> PV matmul with V~ gives
     both the output numerator and the softmax denominator (ones col)
  4. normalize via reciprocal + gpsimd partition_broadcast + DVE mult
  5. AllGather the per-core head outputs (hidden^T) per token chunk
  6. column-parallel FC per chunk: out^T slice = w_fc_slice^T @ hidden^T

All matmuls run in float32r (fp32 data, fast PE mode, ~1e-4 rel err).
"""

import numpy as np

import concourse.bass as bass
import concourse.mybir as mybir
from concourse.tile_rust import add_dep_helper
from concourse.bass_utils import run_bass_kernel_spmd
from concourse.tile import TileContext

# Problem shapes (hardcoded per contract)
B, T, C = 2, 2048, 1024
H, D = 16, 64
N_CORES = 8
HPC = H // N_CORES          # heads per core = 2
HB = HPC * D                # head-block width per core = 128
NT = B * T                  # 4096 tokens
P = 128
TCH = 512                   # token chunk (matmul free dim)
F32 = mybir.dt.float32
F32R = mybir.dt.float32r


def _rope_mats():
    """Per-head [D, D] matrices Rt with q_roped_row = q_row @ Rt (row-vector
    convention), matching reference._rope where the angle is head-dependent
    and position-independent."""
    inv_freq = 1.0 / (10000.0 ** (np.arange(0, D, 2, dtype=np.float64) / D))
    mats = []
    for h in range(H):
        theta = h * inv_freq                      # [D/2]
        c, s = np.cos(theta), np.sin(theta)
        R = np.zeros((D, D), dtype=np.float64)
        R[::2, ::2] = np.diag(c)                  # even <- even*cos
        R[1::2, ::2] = -np.diag(s)                # even <- odd*(-sin)
        R[::2, 1::2] = np.diag(s)                 # odd  <- even*sin
        R[1::2, 1::2] = np.diag(c)                # odd  <- odd*cos
        mats.append(R)
    return mats


def split_sync_commands(nc, max_waits=1, max_updates=1):
    """This container's walrus supports only one sync wait / update per
    instruction. Split excess waits into preceding EventSemaphore instrs on
    the same engine queue, and excess updates into following ones."""
    n_split = 0
    for f in nc.m.functions:
        for bb in f.blocks:
            insts = list(bb.instructions)
            new_list = []
            changed = False
            for inst in insts:
                si = inst.sync_info
                waits = list(si.on_wait) if (si and si.on_wait) else []
                if len(waits) > max_waits:
                    for w in waits[max_waits:]:
                        ev = mybir.InstEventSemaphore(
                            name=f"{inst.name}-wsplit-{n_split}",
                            engine=inst.engine, ins=[], outs=[],
                            sync_info=mybir.SyncInfo(on_wait=[w], on_update=[]),
                        )
                        n_split += 1
                        new_list.append(ev)
                    si.on_wait = waits[:max_waits]
                    changed = True
                new_list.append(inst)
                updates = list(si.on_update) if (si and si.on_update) else []
                if len(updates) > max_updates:
                    opcode = type(inst).__name__
                    if "Dma" in opcode or "DMA" in opcode:
                        raise RuntimeError(
                            f"DMA inst {inst.name} has {len(updates)} updates")
                    si.on_update = updates[:max_updates]
                    for u in updates[max_updates:]:
                        ev = mybir.InstEventSemaphore(
                            name=f"{inst.name}-usplit-{n_split}",
                            engine=inst.engine, ins=[], outs=[],
                            sync_info=mybir.SyncInfo(on_wait=[], on_update=[u]),
                        )
                        n_split += 1
                        new_list.append(ev)
                    changed = True
            if changed:
                bb.instructions = new_list
    return n_split


PHASES = {}


def _rec(phase, inst):
    PHASES[inst.ins.name] = phase
    return inst


def build_kernel():
    nc = bass.Bass(num_devices=N_CORES)

    xT = nc.dram_tensor("xT", [C, NT], F32R, kind="ExternalInput")
    wq = nc.dram_tensor("wq", [C, HB], F32R, kind="ExternalInput")
    wk = nc.dram_tensor("wk", [C, HB], F32R, kind="ExternalInput")
    wv = nc.dram_tensor("wv", [C, HB], F32R, kind="ExternalInput")
    bq = nc.dram_tensor("bq", [HB, 1], F32, kind="ExternalInput")
    bk = nc.dram_tensor("bk", [HB, 1], F32, kind="ExternalInput")
    bv = nc.dram_tensor("bv", [HB, 1], F32, kind="ExternalInput")
    wfc = nc.dram_tensor("wfc", [HB, C], F32R, kind="ExternalInput")
    bfc = nc.dram_tensor("bfc", [HB, 1], F32, kind="ExternalInput")
    ident_in = nc.dram_tensor("ident", [P, P], F32R, kind="ExternalInput")
    ones_in = nc.dram_tensor("ones", [P, D], F32R, kind="ExternalInput")
    # ones64 for the K=1 denominator-broadcast matmul
    outT = nc.dram_tensor("outT", [C, NT], F32, kind="ExternalOutput")

    CB = C // P                      # 8 contraction blocks
    SBLK = T // P                    # 16 s-blocks per batch
    NBC = T // TCH                   # 4 chunks per batch
    Exp = mybir.ActivationFunctionType.Exp

    with TileContext(nc) as tc:
        with (
            tc.tile_pool(name="consts", bufs=1) as consts,
            tc.tile_pool(name="qkv", bufs=1) as qkvp,
            tc.tile_pool(name="work", bufs=2) as work,
            tc.tile_pool(name="expp", bufs=4) as expp,
            tc.tile_pool(name="psum", bufs=1, space="PSUM") as psum,
            tc.tile_pool(name="dram", bufs=1, space="DRAM") as dram,
        ):
            # ---- constants ----
            w_sb = {}
            for name, t in (("wq", wq), ("wk", wk), ("wv", wv)):
                w_t = consts.tile([P, CB, HB], F32R, name=f"{name}_sb")
                nc.sync.dma_start(
                    w_t[:], t[:, :].rearrange("(cb p) o -> p cb o", p=P))
                w_sb[name] = w_t
            wfc_t = consts.tile([P, CB, P], F32R, name="wfc_sb")
            nc.sync.dma_start(
                wfc_t[:], wfc[:, :].rearrange("p (cb o) -> p cb o", o=P))
            w_sb["wfc"] = wfc_t
            b_sb = {}
            for name, t in (("bq", bq), ("bk", bk), ("bv", bv), ("bfc", bfc)):
                b_t = consts.tile([HB, 1], F32, name=f"{name}_sb")
                nc.sync.dma_start(b_t[:], t[:])
                b_sb[name] = b_t
            identity = consts.tile([P, P], F32R, name="identity")
            nc.sync.dma_start(identity[:], ident_in[:])
            ones64 = consts.tile([1, D], F32, name="ones64")
            nc.sync.dma_start(ones64[:], ones_in[0:1, 0:D].bitcast(F32))

            # ---- persistent qkv storage ----
            qT = qkvp.tile([P, NT], F32R, name="qT")
            kz = [qkvp.tile([P, NT], F32R, name=f"kz{h}")
                  for h in range(HPC)]
            vT = qkvp.tile([P, NT], F32R, name="vT")
            # V~: [s-in-block, s-block, 2*(D+1)] with ones at cols D and 2D+1
            vtl = qkvp.tile([P, SBLK * B, 2 * (D + 1)], F32R, name="vtl")
            nc.sync.dma_start(vtl[:, :, D:D + 1],
                              ones_in[:, 0:SBLK * B, None])
            nc.sync.dma_start(vtl[:, :, 2 * D + 1:2 * D + 2],
                              ones_in[:, 0:SBLK * B, None])


            def proj_chunk(tcix):
                tsl = slice(tcix * TCH, (tcix + 1) * TCH)
                xt_tiles = []
                for cb in range(CB):
                    xt = work.tile([P, TCH], F32R, tag="xt", name=f"xt_{cb}",
                                   bufs=2 * CB)
                    nc.sync.dma_start(xt[:], xT[cb * P:(cb + 1) * P, tsl])
                    xt_tiles.append(xt)
                for wname, bname, dst in (("wq", "bq", qT), ("wk", "bk", None),
                                          ("wv", "bv", vT)):
                    ps = psum.tile([P, TCH], F32, tag="mm", name="ps_proj",
                                   bufs=3)
                    for cb in range(CB):
                        _rec(f"proj{tcix}", nc.tensor.matmul(
                            ps[:], w_sb[wname][:, cb, :], xt_tiles[cb],
                            start=(cb == 0), stop=(cb == CB - 1)))
                    if dst is not None:
                        nc.vector.tensor_scalar_add(dst[:, tsl], ps[:],
                                                    b_sb[bname][:])
                    else:
                        # k: write zero-padded per-head copies for full-K ST
                        nc.vector.tensor_scalar_add(
                            kz[0][0:D, tsl], ps[0:D, :], b_sb["bk"][0:D])
                        nc.vector.tensor_scalar_mul(
                            kz[0][D:P, tsl], ps[D:P, :], 0.0)
                        nc.vector.tensor_scalar_add(
                            kz[1][D:P, tsl], ps[D:P, :], b_sb["bk"][D:P])
                        nc.vector.tensor_scalar_mul(
                            kz[1][0:D, tsl], ps[0:D, :], 0.0)

            def vtl_block(sb):
                pst = psum.tile([P, P], F32R, tag="trfc", name="ps_tr",
                                bufs=2)
                _rec(f"vtl{sb}", nc.tensor.transpose(
                    pst[:], vT[:, sb * P:(sb + 1) * P], identity[:]))
                nc.vector.tensor_copy(out=vtl[:, sb, 0:D], in_=pst[:, 0:D])
                nc.vector.tensor_copy(out=vtl[:, sb, D + 1:2 * D + 1],
                                      in_=pst[:, D:2 * D])

            def attention_chunk(b, tcix):
                tsl = slice(b * T + tcix * TCH, b * T + (tcix + 1) * TCH)
                pv_ps = [
                    psum.tile([P, TCH], F32, tag=f"pv{h}",
                              name=f"ps_pv{h}", bufs=(2 if h == 0 else 1))
                    for h in range(HPC)
                ]
                def st_quarter(q):
                    out = []
                    for sb in range(4 * q, 4 * q + 4):
                        ssl = slice(b * T + sb * P, b * T + sb * P + P)
                        for h in range(HPC):
                            ps_st = psum.tile([P, TCH], F32, tag="mm",
                                              name="ps_st", bufs=3)
                            _rec(f"st{b}_{tcix}", nc.tensor.matmul(
                                ps_st[:], kz[h][:, ssl], qT[:, tsl],
                                start=True, stop=True))
                            e = expp.tile([P, TCH], F32R, tag=f"e{h}",
                                          name=f"e{h}", bufs=12)
                            nc.scalar.activation(e[:], ps_st[:], Exp)
                            out.append((sb, h, e))
                    return out

                def pv_quarter(es):
                    lp = None
                    for sb, h, e in es:
                        gsb = b * SBLK + sb
                        lp = _rec(f"pv{b}_{tcix}", nc.tensor.matmul(
                            pv_ps[h][0:D + 1, :],
                            vtl[:, gsb, h * (D + 1):(h + 1) * (D + 1)],
                            e[:],
                            start=(sb == 0), stop=(sb == SBLK - 1)))
                    return lp

                prev_es = None
                for q in range(4):
                    es = st_quarter(q)
                    if prev_es is not None:
                        pv_quarter(prev_es)
                    prev_es = es
                last_pv = pv_quarter(prev_es)
                return pv_ps, last_pv

            def normalize_chunk(b, tcix, pv_ps):
                tsl = slice(b * T + tcix * TCH, b * T + (tcix + 1) * TCH)
                hT = work.tile([P, TCH], F32R, tag="hT", name="hT", bufs=4)
                for h in range(HPC):
                    recip = work.tile([1, TCH], F32, tag="recip",
                                      name="recip", bufs=2)
                    nc.vector.reciprocal(recip[:], pv_ps[h][D:D + 1, :])
                    rb = dram.tile([1, TCH], F32, tag="recip_bounce",
                                   name="rb", bufs=2)
                    nc.sync.dma_start(rb[:], recip[:])
                    bc_sb = work.tile([D, TCH], F32, tag="bc",
                                      name="bc_sb", bufs=2)
                    nc.sync.dma_start(bc_sb[:],
                                      rb[0:1, :].to_broadcast([D, TCH]))
                    nc.vector.tensor_mul(out=hT[h * D:(h + 1) * D, :],
                                         in0=pv_ps[h][0:D, :], in1=bc_sb[:])
                return hT

            def fc_partial(b, tcix, hT, after=None):
                tsl = slice(b * T + tcix * TCH, b * T + (tcix + 1) * TCH)
                for ob in range(CB):
                    ps = psum.tile([P, TCH], F32, tag="trfc", name="ps_fc",
                                   bufs=2)
                    mm = _rec(f"fc{b}_{tcix}", nc.tensor.matmul(
                        ps[:], w_sb["wfc"][:, ob, :], hT[:],
                        start=True, stop=True))
                    if ob == 0 and after is not None:
                        add_dep_helper(mm.ins, after.ins, sync=False,
                                       reason="fc after next attention chunk")
                    osb = work.tile([P, TCH], F32, tag="osb", name="osb",
                                    bufs=3)
                    nc.vector.tensor_copy(out=osb[:], in_=ps[:])
                    nc.sync.dma_start(outT[ob * P:(ob + 1) * P, tsl], osb[:])

            def allgather(key):
                nc.gpsimd.collective_compute(
                    "AllGather", mybir.AluOpType.bypass,
                    replica_groups=[list(range(N_CORES))],
                    ins=[ag_in[key][:].opt()],
                    outs=[ag_out[key][:].opt()])

            z_store = {}

            def fc_load(b, tcix):
                if b == 0:
                    zsrc, otsl = ag_out[0], slice(tcix * TCH,
                                                  (tcix + 1) * TCH)
                elif tcix < 3:
                    zsrc, otsl = ag_out["1a"], slice(tcix * TCH,
                                                     (tcix + 1) * TCH)
                else:
                    zsrc, otsl = ag_out["1b"], slice(0, TCH)
                z_tiles = []
                for cb in range(CB):
                    z = work.tile([P, TCH], F32R, tag="z", name=f"z_{cb}",
                                  bufs=12)
                    nc.sync.dma_start(
                        z[:], zsrc[cb * P:(cb + 1) * P, otsl])
                    z_tiles.append(z)
                z_store[(b, tcix)] = z_tiles

            def fc_mm(b, tcix, after=None):
                tsl = slice(b * T + tcix * TCH, b * T + (tcix + 1) * TCH)
                z_tiles = z_store.pop((b, tcix))
                ps = psum.tile([P, TCH], F32,
                               tag=("trfc" if b == 0 else "pv0"),
                               name="ps_fc", bufs=(1 if b == 0 else 2))
                for cb in range(CB):
                    mm = _rec(f"fc{b}_{tcix}", nc.tensor.matmul(
                        ps[:], w_sb["wfc"][:, cb, :], z_tiles[cb],
                        start=(cb == 0), stop=(cb == CB - 1)))
                    if cb == 0 and after is not None:
                        add_dep_helper(mm.ins, after.ins, sync=False,
                                       reason="fc after attention chunk")
                osb = work.tile([P, TCH], F32, tag="osb", name="osb", bufs=2)
                nc.vector.tensor_scalar_add(osb[:], ps[:], b_sb["bfc"][:])
                nc.sync.dma_start(outT[:, tsl], osb[:])

            # emission order == scheduler priority. proj/FC matmuls act as
            # PE gap-fillers during the exp-bound attention phases.
            for tcix in range(NBC):
                proj_chunk(tcix)
                for sb in range(4 * tcix, 4 * tcix + 4):
                    vtl_block(sb)
            hts0 = {}
            for tcix in range(NBC):
                if tcix >= 2:
                    fc_partial(0, tcix - 2, hts0[tcix - 2])
                pv0, lpv = attention_chunk(0, tcix)
                hts0[tcix] = normalize_chunk(0, tcix, pv0)
            fc_partial(0, NBC - 2, hts0[NBC - 2])
            for tcix in range(NBC, 2 * NBC):
                proj_chunk(tcix)
                for sb in range(4 * tcix, 4 * tcix + 4):
                    vtl_block(sb)
            fc_partial(0, NBC - 1, hts0[NBC - 1])
            hts1 = {}
            for tcix in range(NBC):
                if tcix >= 2:
                    fc_partial(1, tcix - 2, hts1[tcix - 2])
                pv1, lpv = attention_chunk(1, tcix)
                hts1[tcix] = normalize_chunk(1, tcix, pv1)
            fc_partial(1, NBC - 2, hts1[NBC - 2])
            fc_partial(1, NBC - 1, hts1[NBC - 1])

    split_sync_commands(nc)
    return nc


_CACHE = {}


def _prep_inputs(x, w_qkv, b_qkv, w_fc, b_fc):
    """Host-side: fold RoPE + scale into weights, shard per core."""
    rope = _rope_mats()
    w_qkv = np.asarray(w_qkv, dtype=np.float64)
    b_qkv = np.asarray(b_qkv, dtype=np.float64)
    wq_f = w_qkv[:, 0:C].copy()
    wk_f = w_qkv[:, C:2 * C].copy()
    wv_f = w_qkv[:, 2 * C:3 * C].copy()
    bq_f = b_qkv[0:C].copy()
    bk_f = b_qkv[C:2 * C].copy()
    bv_f = b_qkv[2 * C:3 * C].copy()
    scale = 1.0 / np.sqrt(D)
    for h in range(H):
        sl = slice(h * D, (h + 1) * D)
        wq_f[:, sl] = (wq_f[:, sl] @ rope[h]) * scale
        bq_f[sl] = (bq_f[sl] @ rope[h]) * scale
        wk_f[:, sl] = wk_f[:, sl] @ rope[h]
        bk_f[sl] = bk_f[sl] @ rope[h]

    xT = np.ascontiguousarray(
        np.asarray(x, dtype=np.float32).reshape(NT, C).T)

    in_maps = []
    for m in range(N_CORES):
        sl = slice(m * HB, (m + 1) * HB)
        in_maps.append({
            "xT": xT,
            "wq": np.ascontiguousarray(wq_f[:, sl], dtype=np.float32),
            "wk": np.ascontiguousarray(wk_f[:, sl], dtype=np.float32),
            "wv": np.ascontiguousarray(wv_f[:, sl], dtype=np.float32),
            "bq": np.ascontiguousarray(bq_f[sl, None], dtype=np.float32),
            "bk": np.ascontiguousarray(bk_f[sl, None], dtype=np.float32),
            "bv": np.ascontiguousarray(bv_f[sl, None], dtype=np.float32),
            "wfc": np.ascontiguousarray(np.asarray(w_fc)[sl, :], dtype=np.float32),
            "bfc": np.ascontiguousarray(
                np.asarray(b_fc, dtype=np.float32)[sl, None]),
            "ident": np.eye(P, dtype=np.float32),
            "ones": np.ones((P, D), dtype=np.float32),
        })
    return in_maps


def kernel(x, w_qkv, b_qkv, w_fc, b_fc, _trace=False):
    in_maps = _prep_inputs(x, w_qkv, b_qkv, w_fc, b_fc)
    if "nc" not in _CACHE:
        _CACHE["nc"] = build_kernel()
    nc = _CACHE["nc"]
    res = run_bass_kernel_spmd(nc, in_maps, core_ids=list(range(N_CORES)),
                               trace=_trace)
    _CACHE["last_result"] = res
    acc = res.results[0]["outT"].astype(np.float64)
    for m in range(1, N_CORES):
        acc += res.results[m]["outT"]
    out = acc.T + np.asarray(b_fc, dtype=np.float64)[None, :]
    return np.ascontiguousarray(out.reshape(B, T, C).astype(np.float32))



# revision 3
# speedup vs baseline: 1.1712x; 1.1712x over previous
"""Multi-head self-attention (dense transformer block) on 8 Trainium2 cores.

Tensor-parallel over heads: core m handles heads {2m, 2m+1} for both batch
elements. The reference's RoPE uses angles that depend only on the head
index (not the position), so it is a fixed orthogonal rotation per head;
we fold it (and the 1/sqrt(D) score scale) into the QKV weights on the
host. The V bias commutes through softmax (sum of weights = 1) and the FC
matmul, so it is folded into the host-side output bias: b_eff = b_fc +
w_fc^T b_v. All device operands are fp16 (PSUM accumulation fp32); the
2e-2 rel-err budget has orders of magnitude of headroom.

Device pipeline per core:
  1. qT/kp = w^T x^T  [d-major, 2 heads stacked on partitions 0:64/64:128]
     V~ computed directly token-major: per 128-token block,
     out[s, d] = sum_c x[c, s] wv[c, d], written into the [s, 2*(D+1)]
     V~ layout with a ones column per head (PV denominator trick).
  2. per (batch, 512-token chunk): for each 128-key block, TWO row-tiled
     K=64 matmuls (one per head, PE rows 0:63 / 64:127) produce both
     heads' score tiles into one 2-bank PSUM pair; ONE exp activation
     (N=1024) turns the pair into fp16 e-tiles; per-head PV matmuls
     accumulate numerator + denominator (ones column) in PSUM.
  3. normalize: reciprocal_approx_fast on the denominator row, DMA
     broadcast across partitions, fused multiply -> hT (fp16).
  4. row-parallel FC: partial out^T = w_fc_slice^T hT per core, written
     fp16; host sums the 8 partials and adds b_eff.
"""

import numpy as np

import concourse.bass as bass
import concourse.mybir as mybir
from concourse.bass_utils import run_bass_kernel_spmd
from concourse.tile import TileContext

# Problem shapes (hardcoded per contract)
B, T, C = 2, 2048, 1024
H, D = 16, 64
N_CORES = 8
HPC = H // N_CORES          # heads per core = 2
HB = HPC * D                # head-block width per core = 128
NT = B * T                  # 4096 tokens
P = 128
TCH = 512                   # token chunk (matmul free dim)
SBLK = T // P               # 16 key blocks per batch
NBC = T // TCH              # 4 chunks per batch
CB = C // P                 # 8 contraction blocks
F32 = mybir.dt.float32
F16 = mybir.dt.float16


def _rope_mats():
    """Per-head [D, D] matrices Rt with q_roped_row = q_row @ Rt (row-vector
    convention), matching reference._rope where the angle is head-dependent
    and position-independent."""
    inv_freq = 1.0 / (10000.0 ** (np.arange(0, D, 2, dtype=np.float64) / D))
    mats = []
    for h in range(H):
        theta = h * inv_freq                      # [D/2]
        c, s = np.cos(theta), np.sin(theta)
        R = np.zeros((D, D), dtype=np.float64)
        R[::2, ::2] = np.diag(c)                  # even <- even*cos
        R[1::2, ::2] = -np.diag(s)                # even <- odd*(-sin)
        R[::2, 1::2] = np.diag(s)                 # odd  <- even*sin
        R[1::2, 1::2] = np.diag(c)                # odd  <- odd*cos
        mats.append(R)
    return mats


def split_sync_commands(nc, max_waits=1, max_updates=1):
    """This container's walrus supports only one sync wait / update per
    instruction. Split excess waits into preceding EventSemaphore instrs on
    the same engine queue, and excess updates into following ones."""
    n_split = 0
    for f in nc.m.functions:
        for bb in f.blocks:
            insts = list(bb.instructions)
            new_list = []
            changed = False
            for inst in insts:
                si = inst.sync_info
                waits = list(si.on_wait) if (si and si.on_wait) else []
                if len(waits) > max_waits:
                    for w in waits[max_waits:]:
                        ev = mybir.InstEventSemaphore(
                            name=f"{inst.name}-wsplit-{n_split}",
                            engine=inst.engine, ins=[], outs=[],
                            sync_info=mybir.SyncInfo(on_wait=[w], on_update=[]),
                        )
                        n_split += 1
                        new_list.append(ev)
                    si.on_wait = waits[:max_waits]
                    changed = True
                new_list.append(inst)
                updates = list(si.on_update) if (si and si.on_update) else []
                if len(updates) > max_updates:
                    opcode = type(inst).__name__
                    if "Dma" in opcode or "DMA" in opcode:
                        raise RuntimeError(
                            f"DMA inst {inst.name} has {len(updates)} updates")
                    si.on_update = updates[:max_updates]
                    for u in updates[max_updates:]:
                        ev = mybir.InstEventSemaphore(
                            name=f"{inst.name}-usplit-{n_split}",
                            engine=inst.engine, ins=[], outs=[],
                            sync_info=mybir.SyncInfo(on_wait=[], on_update=[u]),
                        )
                        n_split += 1
                        new_list.append(ev)
                    changed = True
            if changed:
                bb.instructions = new_list
    return n_split


PHASES = {}


def _rec(phase, inst):
    PHASES[inst.ins.name] = phase
    return inst


def build_kernel():
    nc = bass.Bass(num_devices=N_CORES)

    xT = nc.dram_tensor("xT", [C, NT], F16, kind="ExternalInput")
    wq = nc.dram_tensor("wq", [C, HB], F16, kind="ExternalInput")
    wk = nc.dram_tensor("wk", [C, HB], F16, kind="ExternalInput")
    wv = nc.dram_tensor("wv", [C, HB], F16, kind="ExternalInput")
    bq = nc.dram_tensor("bq", [HB, 1], F32, kind="ExternalInput")
    bk = nc.dram_tensor("bk", [HB, 1], F32, kind="ExternalInput")
    wfc = nc.dram_tensor("wfc", [HB, C], F16, kind="ExternalInput")
    outT = nc.dram_tensor("outT", [C, NT], F16, kind="ExternalOutput")

    Exp = mybir.ActivationFunctionType.Exp

    with TileContext(nc) as tc:
        with (
            tc.tile_pool(name="consts", bufs=1) as consts,
            tc.tile_pool(name="qkv", bufs=1) as qkvp,
            tc.tile_pool(name="work", bufs=2) as work,
            tc.tile_pool(name="expp", bufs=6) as expp,
            tc.tile_pool(name="psum", bufs=1, space="PSUM") as psum,
            tc.tile_pool(name="dram", bufs=1, space="DRAM") as dram,
        ):
            # ---- constants ----
            w_sb = {}
            for name, t in (("wq", wq), ("wk", wk), ("wv", wv)):
                w_t = consts.tile([P, CB, HB], F16, name=f"{name}_sb")
                nc.sync.dma_start(
                    w_t[:], t[:, :].rearrange("(cb p) o -> p cb o", p=P))
                w_sb[name] = w_t
            wfc_t = consts.tile([P, CB, P], F16, name="wfc_sb")
            nc.sync.dma_start(
                wfc_t[:], wfc[:, :].rearrange("p (cb o) -> p cb o", o=P))
            w_sb["wfc"] = wfc_t
            b_sb = {}
            for name, t in (("bq", bq), ("bk", bk)):
                b_t = consts.tile([HB, 1], F32, name=f"{name}_sb")
                nc.sync.dma_start(b_t[:], t[:])
                b_sb[name] = b_t

            # ---- persistent qkv storage ----
            qT = qkvp.tile([P, NT], F16, name="qT")
            kp = qkvp.tile([P, NT], F16, name="kp")
            # V~: [s-in-block, s-block, 2*(D+1)] with ones at cols D and 2D+1
            vtl = qkvp.tile([P, SBLK * B, 2 * (D + 1)], F16, name="vtl")
            nc.vector.memset(vtl[:, :, D:D + 1], 1.0)
            nc.vector.memset(vtl[:, :, 2 * D + 1:2 * D + 2], 1.0)

            def proj_chunk(ci):
                """Global 512-token chunk ci: q/k d-major + V~ token-major."""
                tsl = slice(ci * TCH, (ci + 1) * TCH)
                xt = work.tile([P, CB, TCH], F16, tag="xt", name="xt", bufs=2)
                nc.sync.dma_start(
                    xt[:], xT[:, tsl].rearrange("(cb p) t -> p cb t", p=P))
                for wname, bname, dst in (("wq", "bq", qT), ("wk", "bk", kp)):
                    ps = psum.tile([P, TCH], F32, tag="mm", name="ps_proj",
                                   bufs=2)
                    for cb in range(CB):
                        _rec(f"proj{ci}", nc.tensor.matmul(
                            ps[:], w_sb[wname][:, cb, :], xt[:, cb, :],
                            start=(cb == 0), stop=(cb == CB - 1)))
                    nc.vector.tensor_scalar_add(dst[:, tsl], ps[:],
                                                b_sb[bname][:])
                # V~ directly token-major: out[s, d] = sum_c x[c, s] wv[c, d]
                for j in range(4):
                    gsb = ci * 4 + j
                    psv = psum.tile([P, TCH], F32, tag="mm", name="ps_v",
                                    bufs=2)
                    for cb in range(CB):
                        _rec(f"proj{ci}", nc.tensor.matmul(
                            psv[:, 0:P], xt[:, cb, j * P:(j + 1) * P],
                            w_sb["wv"][:, cb, :],
                            start=(cb == 0), stop=(cb == CB - 1)))
                    nc.vector.tensor_copy(
                        out=vtl[:, gsb, :].rearrange(
                            "p (h x) -> p h x", h=2)[:, :, 0:D],
                        in_=psv[:, 0:P].rearrange("p (h d) -> p h d", h=2))

            def attention_chunk(b, tcix):
                tsl = slice(b * T + tcix * TCH, b * T + (tcix + 1) * TCH)
                pv_ps = [
                    psum.tile([P, TCH], F32, tag=f"pv{h}", name=f"ps_pv{h}",
                              bufs=1)
                    for h in range(HPC)
                ]
                pend = None   # (gsb, e) awaiting PV

                def do_pv(gsb, e):
                    for h in range(HPC):
                        _rec(f"pv{b}_{tcix}", nc.tensor.matmul(
                            pv_ps[h][0:D + 1, :],
                            vtl[:, gsb, h * (D + 1):(h + 1) * (D + 1)],
                            e[:, h * TCH:(h + 1) * TCH],
                            start=(gsb % SBLK == 0),
                            stop=(gsb % SBLK == SBLK - 1)))

                for g in range(SBLK):
                    gsb = b * SBLK + g
                    ssl = slice(b * T + g * P, b * T + g * P + P)
                    stp = psum.tile([P, 2 * TCH], F32, tag="st", name="stp",
                                    bufs=2)
                    for h in range(HPC):
                        _rec(f"st{b}_{tcix}", nc.tensor.matmul(
                            stp[:, h * TCH:(h + 1) * TCH],
                            kp[h * D:(h + 1) * D, ssl],
                            qT[h * D:(h + 1) * D, tsl],
                            start=True, stop=True))
                    e = expp.tile([P, 2 * TCH], F16, tag="e", name="e",
                                  bufs=6)
                    nc.scalar.activation(e[:], stp[:], Exp)
                    if pend is not None:
                        do_pv(*pend)
                    pend = (gsb, e)
                do_pv(*pend)
                return pv_ps

            def normalize_chunk(b, tcix, pv_ps):
                hT = work.tile([P, TCH], F16, tag="hT", name="hT", bufs=2)
                for h in range(HPC):
                    recip = work.tile([1, TCH], F32, tag="recip",
                                      name="recip", bufs=2)
                    nc.vector.reciprocal(recip[:], pv_ps[h][D:D + 1, :])
                    rb = dram.tile([1, TCH], F32, tag="recip_bounce",
                                   name="rb", bufs=2)
                    nc.sync.dma_start(rb[:], recip[:])
                    bc_sb = work.tile([D, TCH], F32, tag="bc",
                                      name="bc_sb", bufs=2)
                    nc.sync.dma_start(bc_sb[:],
                                      rb[0:1, :].to_broadcast([D, TCH]))
                    nc.vector.tensor_mul(out=hT[h * D:(h + 1) * D, :],
                                         in0=pv_ps[h][0:D, :], in1=bc_sb[:])
                return hT

            def fc_chunk(b, tcix, hT):
                tsl = slice(b * T + tcix * TCH, b * T + (tcix + 1) * TCH)
                osb = work.tile([P, CB, TCH], F16, tag="osb", name="osb",
                                bufs=2)
                for ob in range(CB):
                    ps = psum.tile([P, TCH], F32, tag="mm", name="ps_fc",
                                   bufs=2)
                    _rec(f"fc{b}_{tcix}", nc.tensor.matmul(
                        ps[:], w_sb["wfc"][:, ob, :], hT[:],
                        start=True, stop=True))
                    nc.vector.tensor_copy(out=osb[:, ob, :], in_=ps[:])
                nc.sync.dma_start(
                    outT[:, tsl].rearrange("(ob p) t -> p ob t", p=P),
                    osb[:])

            # emission order == scheduler priority.
            for t in range(NBC):
                proj_chunk(t)                      # batch 0
            hts = {}
            pv = attention_chunk(0, 0)
            hts[0] = normalize_chunk(0, 0, pv)
            for t in range(1, NBC):
                pv = attention_chunk(0, t)
                hts[t] = normalize_chunk(0, t, pv)
                proj_chunk(NBC + t - 1)            # batch 1 chunks 0..2
                fc_chunk(0, t - 1, hts[t - 1])
            proj_chunk(2 * NBC - 1)                # batch 1 last chunk
            fc_chunk(0, NBC - 1, hts[NBC - 1])
            pv = attention_chunk(1, 0)
            hts[0] = normalize_chunk(1, 0, pv)
            for t in range(1, NBC):
                pv = attention_chunk(1, t)
                hts[t] = normalize_chunk(1, t, pv)
                fc_chunk(1, t - 1, hts[t - 1])
            fc_chunk(1, NBC - 1, hts[NBC - 1])

    split_sync_commands(nc)
    return nc


_CACHE = {}


def _prep_inputs(x, w_qkv, b_qkv, w_fc, b_fc):
    """Host-side: fold RoPE + scale into weights, fold V bias into the
    output bias, shard per core, cast to fp16."""
    rope = _rope_mats()
    w_qkv = np.asarray(w_qkv, dtype=np.float64)
    b_qkv = np.asarray(b_qkv, dtype=np.float64)
    w_fc64 = np.asarray(w_fc, dtype=np.float64)
    wq_f = w_qkv[:, 0:C].copy()
    wk_f = w_qkv[:, C:2 * C].copy()
    wv_f = w_qkv[:, 2 * C:3 * C].copy()
    bq_f = b_qkv[0:C].copy()
    bk_f = b_qkv[C:2 * C].copy()
    bv_f = b_qkv[2 * C:3 * C].copy()
    scale = 1.0 / np.sqrt(D)
    for h in range(H):
        sl = slice(h * D, (h + 1) * D)
        wq_f[:, sl] = (wq_f[:, sl] @ rope[h]) * scale
        bq_f[sl] = (bq_f[sl] @ rope[h]) * scale
        wk_f[:, sl] = wk_f[:, sl] @ rope[h]
        bk_f[sl] = bk_f[sl] @ rope[h]

    # V bias commutes through softmax (weights sum to 1) and the FC matmul.
    b_eff = np.asarray(b_fc, dtype=np.float64) + bv_f @ w_fc64

    xT = np.ascontiguousarray(
        np.asarray(x, dtype=np.float16).reshape(NT, C).T)

    in_maps = []
    for m in range(N_CORES):
        sl = slice(m * HB, (m + 1) * HB)
        in_maps.append({
            "xT": xT,
            "wq": np.ascontiguousarray(wq_f[:, sl], dtype=np.float16),
            "wk": np.ascontiguousarray(wk_f[:, sl], dtype=np.float16),
            "wv": np.ascontiguousarray(wv_f[:, sl], dtype=np.float16),
            "bq": np.ascontiguousarray(bq_f[sl, None], dtype=np.float32),
            "bk": np.ascontiguousarray(bk_f[sl, None], dtype=np.float32),
            "wfc": np.ascontiguousarray(w_fc64[sl, :], dtype=np.float16),
        })
    return in_maps, b_eff


def kernel(x, w_qkv, b_qkv, w_fc, b_fc, _trace=False):
    in_maps, b_eff = _prep_inputs(x, w_qkv, b_qkv, w_fc, b_fc)
    if "nc" not in _CACHE:
        _CACHE["nc"] = build_kernel()
    nc = _CACHE["nc"]
    res = run_bass_kernel_spmd(nc, in_maps, core_ids=list(range(N_CORES)),
                               trace=_trace)
    _CACHE["last_result"] = res
    acc = res.results[0]["outT"].astype(np.float64)
    for m in range(1, N_CORES):
        acc += res.results[m]["outT"]
    out = acc.T + b_eff[None, :]
    return np.ascontiguousarray(out.reshape(B, T, C).astype(np.float32))


# revision 6
# speedup vs baseline: 1.1895x; 1.0156x over previous
"""Multi-head self-attention (dense transformer block) on 8 Trainium2 cores.

Tensor-parallel over heads: core m handles heads {2m, 2m+1} for both batch
elements. The reference's RoPE uses angles that depend only on the head
index (not the position), so it is a fixed orthogonal rotation per head;
we fold it (and the 1/sqrt(D) score scale) into the QKV weights on the
host. The V bias commutes through softmax (sum of weights = 1) and the FC
matmul, so it is folded into the host-side output bias: b_eff = b_fc +
w_fc^T b_v. All device operands are fp16 (PSUM accumulation fp32); the
2e-2 rel-err budget has orders of magnitude of headroom.

Device pipeline per core:
  1. qT/kp = w^T x^T  [d-major, 2 heads stacked on partitions 0:64/64:128]
     V~ computed directly token-major: per 128-token block,
     out[s, d] = sum_c x[c, s] wv[c, d], written into the [s, 2*(D+1)]
     V~ layout with a ones column per head (PV denominator trick).
  2. per (batch, 512-token chunk): for each 128-key block, TWO row-tiled
     K=64 matmuls (one per head, PE rows 0:63 / 64:127) produce both
     heads' score tiles into one 2-bank PSUM pair; ONE exp activation
     (N=1024) turns the pair into fp16 e-tiles; per-head PV matmuls
     accumulate numerator + denominator (ones column) in PSUM.
     Emission is software-pipelined: ST/exp of iteration i, PV of i-1,
     plus up to two "filler" units (next batch's projection, previous
     chunk's FC) per iteration so the PE stream never blocks the ACT
     engine for long.
  3. normalize: reciprocal on the Scalar engine as exp(-ln(d)) (both
     functions live in one ACT table set), DMA broadcast across
     partitions, multiply -> hT (fp16).
  4. row-parallel FC: partial out^T = w_fc_slice^T hT per core, written
     fp16; host sums the 8 partials and adds b_eff.
"""

from collections import deque

import numpy as np

import concourse.bass as bass
import concourse.bass_utils as _bass_utils
import concourse.mybir as mybir
from concourse.bass_utils import run_bass_kernel_spmd
from concourse.tile import TileContext


# Problem shapes (hardcoded per contract)
B, T, C = 2, 2048, 1024
H, D = 16, 64
N_CORES = 8
HPC = H // N_CORES          # heads per core = 2
HB = HPC * D                # head-block width per core = 128
NT = B * T                  # 4096 tokens
P = 128
TCH = 512                   # token chunk (matmul free dim)
SBLK = T // P               # 16 key blocks per batch
NBC = T // TCH              # 4 chunks per batch
CB = C // P                 # 8 contraction blocks
F32 = mybir.dt.float32
F16 = mybir.dt.float16


def _rope_mats():
    """Per-head [D, D] matrices Rt with q_roped_row = q_row @ Rt (row-vector
    convention), matching reference._rope where the angle is head-dependent
    and position-independent."""
    inv_freq = 1.0 / (10000.0 ** (np.arange(0, D, 2, dtype=np.float64) / D))
    mats = []
    for h in range(H):
        theta = h * inv_freq                      # [D/2]
        c, s = np.cos(theta), np.sin(theta)
        R = np.zeros((D, D), dtype=np.float64)
        R[::2, ::2] = np.diag(c)                  # even <- even*cos
        R[1::2, ::2] = -np.diag(s)                # even <- odd*(-sin)
        R[::2, 1::2] = np.diag(s)                 # odd  <- even*sin
        R[1::2, 1::2] = np.diag(c)                # odd  <- odd*cos
        mats.append(R)
    return mats


def split_sync_commands(nc, max_waits=1, max_updates=1):
    """This container's walrus supports only one sync wait / update per
    instruction. Split excess waits into preceding EventSemaphore instrs on
    the same engine queue, and excess updates into following ones."""
    n_split = 0
    for f in nc.m.functions:
        for bb in f.blocks:
            insts = list(bb.instructions)
            new_list = []
            changed = False
            for inst in insts:
                si = inst.sync_info
                waits = list(si.on_wait) if (si and si.on_wait) else []
                if len(waits) > max_waits:
                    for w in waits[max_waits:]:
                        ev = mybir.InstEventSemaphore(
                            name=f"{inst.name}-wsplit-{n_split}",
                            engine=inst.engine, ins=[], outs=[],
                            sync_info=mybir.SyncInfo(on_wait=[w], on_update=[]),
                        )
                        n_split += 1
                        new_list.append(ev)
                    si.on_wait = waits[:max_waits]
                    changed = True
                new_list.append(inst)
                updates = list(si.on_update) if (si and si.on_update) else []
                if len(updates) > max_updates:
                    opcode = type(inst).__name__
                    if "Dma" in opcode or "DMA" in opcode:
                        raise RuntimeError(
                            f"DMA inst {inst.name} has {len(updates)} updates")
                    si.on_update = updates[:max_updates]
                    for u in updates[max_updates:]:
                        ev = mybir.InstEventSemaphore(
                            name=f"{inst.name}-usplit-{n_split}",
                            engine=inst.engine, ins=[], outs=[],
                            sync_info=mybir.SyncInfo(on_wait=[], on_update=[u]),
                        )
                        n_split += 1
                        new_list.append(ev)
                    changed = True
            if changed:
                bb.instructions = new_list
    return n_split


PHASES = {}


def _rec(phase, inst):
    PHASES[inst.ins.name] = phase
    return inst


def build_kernel():
    nc = bass.Bass(num_devices=N_CORES)

    xT = nc.dram_tensor("xT", [C, NT], F16, kind="ExternalInput")
    wq = nc.dram_tensor("wq", [C, HB], F16, kind="ExternalInput")
    wk = nc.dram_tensor("wk", [C, HB], F16, kind="ExternalInput")
    wv = nc.dram_tensor("wv", [C, HB], F16, kind="ExternalInput")
    bq = nc.dram_tensor("bq", [HB, 1], F32, kind="ExternalInput")
    bk = nc.dram_tensor("bk", [HB, 1], F32, kind="ExternalInput")
    wfc = nc.dram_tensor("wfc", [HB, C], F16, kind="ExternalInput")
    outT = nc.dram_tensor("outT", [C, NT], F16, kind="ExternalOutput")

    Exp = mybir.ActivationFunctionType.Exp
    Ln = mybir.ActivationFunctionType.Ln

    with TileContext(nc) as tc:
        with (
            tc.tile_pool(name="consts", bufs=1) as consts,
            tc.tile_pool(name="qkv", bufs=1) as qkvp,
            tc.tile_pool(name="work", bufs=2) as work,
            tc.tile_pool(name="expp", bufs=10) as expp,
            tc.tile_pool(name="psum", bufs=1, space="PSUM") as psum,
            tc.tile_pool(name="dram", bufs=1, space="DRAM") as dram,
        ):
            # ---- constants (weight loads on the ACT DMA queue so they run
            # in parallel with the first x chunks on the sync queue) ----
            w_sb = {}
            for name, t in (("wq", wq), ("wk", wk), ("wv", wv)):
                w_t = consts.tile([P, CB, HB], F16, name=f"{name}_sb")
                nc.scalar.dma_start(
                    w_t[:], t[:, :].rearrange("(cb p) o -> p cb o", p=P))
                w_sb[name] = w_t
            wfc_t = consts.tile([P, CB, P], F16, name="wfc_sb")
            nc.scalar.dma_start(
                wfc_t[:], wfc[:, :].rearrange("p (cb o) -> p cb o", o=P))
            w_sb["wfc"] = wfc_t
            b_sb = {}
            for name, t in (("bq", bq), ("bk", bk)):
                b_t = consts.tile([HB, 1], F32, name=f"{name}_sb")
                nc.scalar.dma_start(b_t[:], t[:])
                b_sb[name] = b_t

            # ---- persistent qkv storage ----
            qT = qkvp.tile([P, NT], F16, name="qT")
            kp = qkvp.tile([P, NT], F16, name="kp")
            # V~: [s-in-block, s-block, 2*(D+1)] with ones at cols D and 2D+1
            vtl = qkvp.tile([P, SBLK * B, 2 * (D + 1)], F16, name="vtl")
            nc.vector.memset(vtl[:, :, D:D + 1], 1.0)
            nc.vector.memset(vtl[:, :, 2 * D + 1:2 * D + 2], 1.0)

            filler_q = deque()

            def pump(n):
                c = 0
                while filler_q and c < n:
                    filler_q.popleft()()
                    c += 1

            def proj_units(ci):
                """Emit projection of global 512-token chunk ci as filler
                units: xt DMA, q (2 units), k (2 units), V~ (4 units)."""
                tsl = slice(ci * TCH, (ci + 1) * TCH)
                st = {}

                def u_dma():
                    xt = work.tile([P, CB, TCH], F16, tag="xt", name="xt",
                                   bufs=4)
                    nc.sync.dma_start(
                        xt[:], xT[:, tsl].rearrange("(cb p) t -> p cb t", p=P))
                    st["xt"] = xt

                def mk_qk(wname, bname, dst, lo, hi):
                    def u():
                        if lo == 0:
                            st[wname] = psum.tile([P, TCH], F32, tag="mm",
                                                  name="ps_proj", bufs=2)
                        ps = st[wname]
                        for cb in range(lo, hi):
                            _rec(f"proj{ci}", nc.tensor.matmul(
                                ps[:], w_sb[wname][:, cb, :],
                                st["xt"][:, cb, :],
                                start=(cb == 0), stop=(cb == CB - 1)))
                        if hi == CB:
                            nc.vector.tensor_scalar_add(
                                dst[:, tsl], ps[:], b_sb[bname][:])
                    return u

                def mk_v(j):
                    def u():
                        gsb = ci * 4 + j
                        psv = psum.tile([P, TCH], F32, tag="mm", name="ps_v",
                                        bufs=2)
                        for cb in range(CB):
                            _rec(f"proj{ci}", nc.tensor.matmul(
                                psv[:, 0:P],
                                st["xt"][:, cb, j * P:(j + 1) * P],
                                w_sb["wv"][:, cb, :],
                                start=(cb == 0), stop=(cb == CB - 1)))
                        nc.vector.tensor_copy(
                            out=vtl[:, gsb, :].rearrange(
                                "p (h x) -> p h x", h=2)[:, :, 0:D],
                            in_=psv[:, 0:P].rearrange("p (h d) -> p h d", h=2))
                    return u

                units = [u_dma,
                         mk_qk("wq", "bq", qT, 0, 4),
                         mk_qk("wq", "bq", qT, 4, CB),
                         mk_qk("wk", "bk", kp, 0, 4),
                         mk_qk("wk", "bk", kp, 4, CB),
                         mk_v(0), mk_v(1), mk_v(2), mk_v(3)]
                return units

            def normalize_chunk(b, tcix, pv_ps):
                """ACT reciprocal (exp(-ln d)), broadcast, multiply."""
                hT = work.tile([P, TCH], F16, tag="hT", name="hT", bufs=2)
                for h in range(HPC):
                    lnd = work.tile([1, TCH], F32, tag="lnd", name="lnd",
                                    bufs=2)
                    nc.scalar.activation(lnd[:], pv_ps[h][D:D + 1, :], Ln)
                    recip = work.tile([1, TCH], F32, tag="recip",
                                      name="recip", bufs=2)
                    nc.scalar.activation(recip[:], lnd[:], Exp, scale=-1.0)
                    rb = dram.tile([1, TCH], F32, tag="recip_bounce",
                                   name="rb", bufs=2)
                    nc.sync.dma_start(rb[:], recip[:])
                    bc_sb = work.tile([D, TCH], F32, tag="bc",
                                      name="bc_sb", bufs=2)
                    nc.sync.dma_start(bc_sb[:],
                                      rb[0:1, :].to_broadcast([D, TCH]))
                    nc.vector.tensor_mul(out=hT[h * D:(h + 1) * D, :],
                                         in0=pv_ps[h][0:D, :], in1=bc_sb[:])
                return hT

            def fc_units(b, tcix, hT_ref):
                """FC of chunk (b, tcix) as filler units: one MM+copy per
                output block, output DMA split in two."""
                tsl = slice(b * T + tcix * TCH, b * T + (tcix + 1) * TCH)
                st = {}

                def u_alloc():
                    st["osb"] = work.tile([P, CB, TCH], F16, tag="osb",
                                          name="osb", bufs=2)

                def mk_ob(ob):
                    def u():
                        ps = psum.tile([P, TCH], F32, tag="mm", name="ps_fc",
                                       bufs=2)
                        _rec(f"fc{b}_{tcix}", nc.tensor.matmul(
                            ps[:], w_sb["wfc"][:, ob, :], hT_ref[0][:],
                            start=True, stop=True))
                        nc.vector.tensor_copy(out=st["osb"][:, ob, :],
                                              in_=ps[:])
                        if ob == CB // 2 - 1 or ob == CB - 1:
                            half = slice(0, CB // 2) if ob < CB // 2 else \
                                slice(CB // 2, CB)
                            otsl = slice(tsl.start, tsl.stop)
                            nc.sync.dma_start(
                                outT[half.start * P:half.stop * P, otsl]
                                .rearrange("(ob p) t -> p ob t", p=P),
                                st["osb"][:, half, :])
                    return u

                return [u_alloc] + [mk_ob(ob) for ob in range(CB)]

            def attention_batch(b):
                """Software-pipelined attention over all chunks of batch b.
                Iteration (tcix, g): ST pair + exp of (tcix, g), PV of the
                previous iteration, plus up to 2 filler units."""
                pend = None           # (pv_ps, gsb, e, tcix, is_last)
                hts = {}
                pv_ps = None

                def do_pv(pv_ps_, gsb, e, tcix_, last):
                    g = gsb % SBLK
                    for h in range(HPC):
                        _rec(f"pv{b}_{tcix_}", nc.tensor.matmul(
                            pv_ps_[h][0:D + 1, :],
                            vtl[:, gsb, h * (D + 1):(h + 1) * (D + 1)],
                            e[:, h * TCH:(h + 1) * TCH],
                            start=(g == 0), stop=(g == SBLK - 1)))
                    if last:
                        hts[tcix_] = normalize_chunk(b, tcix_, pv_ps_)
                        if b == 0 or tcix_ < NBC - 1:
                            filler_q.extend(
                                fc_units(b, tcix_, [hts[tcix_]]))

                for tcix in range(NBC):
                    tsl = slice(b * T + tcix * TCH, b * T + (tcix + 1) * TCH)
                    pv_ps = [
                        psum.tile([P, TCH], F32, tag=f"pv{h}",
                                  name=f"ps_pv{h}", bufs=1)
                        for h in range(HPC)
                    ]
                    for g in range(SBLK):
                        gsb = b * SBLK + g
                        ssl = slice(b * T + g * P, b * T + g * P + P)
                        stp = psum.tile([P, 2 * TCH], F32, tag="st",
                                        name="stp", bufs=2)
                        for h in range(HPC):
                            _rec(f"st{b}_{tcix}", nc.tensor.matmul(
                                stp[:, h * TCH:(h + 1) * TCH],
                                kp[h * D:(h + 1) * D, ssl],
                                qT[h * D:(h + 1) * D, tsl],
                                start=True, stop=True))
                        e = expp.tile([P, 2 * TCH], F16, tag="e", name="e",
                                      bufs=10)
                        nc.scalar.activation(e[:], stp[:], Exp)
                        if pend is not None:
                            do_pv(*pend)
                        pend = (pv_ps, gsb, e, tcix,
                                g == SBLK - 1)
                        pump(2)
                do_pv(*pend)
                return hts

            # ---- emission order == scheduler priority ----
            for t in range(NBC):                   # batch 0 projection
                for u in proj_units(t):
                    u()
            for t in range(NBC):                   # batch 1 proj as fillers
                filler_q.extend(proj_units(NBC + t))
            hts0 = attention_batch(0)
            pump(len(filler_q))                    # drain leftovers
            hts1 = attention_batch(1)
            pump(len(filler_q))
            for u in fc_units(1, NBC - 1, [hts1[NBC - 1]]):
                u()

    split_sync_commands(nc)
    return nc


_CACHE = {}


def _prep_inputs(x, w_qkv, b_qkv, w_fc, b_fc):
    """Host-side: fold RoPE + scale into weights, fold V bias into the
    output bias, shard per core, cast to fp16."""
    rope = _rope_mats()
    w_qkv = np.asarray(w_qkv, dtype=np.float64)
    b_qkv = np.asarray(b_qkv, dtype=np.float64)
    w_fc64 = np.asarray(w_fc, dtype=np.float64)
    wq_f = w_qkv[:, 0:C].copy()
    wk_f = w_qkv[:, C:2 * C].copy()
    wv_f = w_qkv[:, 2 * C:3 * C].copy()
    bq_f = b_qkv[0:C].copy()
    bk_f = b_qkv[C:2 * C].copy()
    bv_f = b_qkv[2 * C:3 * C].copy()
    scale = 1.0 / np.sqrt(D)
    for h in range(H):
        sl = slice(h * D, (h + 1) * D)
        wq_f[:, sl] = (wq_f[:, sl] @ rope[h]) * scale
        bq_f[sl] = (bq_f[sl] @ rope[h]) * scale
        wk_f[:, sl] = wk_f[:, sl] @ rope[h]
        bk_f[sl] = bk_f[sl] @ rope[h]

    # V bias commutes through softmax (weights sum to 1) and the FC matmul.
    b_eff = np.asarray(b_fc, dtype=np.float64) + bv_f @ w_fc64

    xT = np.ascontiguousarray(
        np.asarray(x, dtype=np.float16).reshape(NT, C).T)

    in_maps = []
    for m in range(N_CORES):
        sl = slice(m * HB, (m + 1) * HB)
        in_maps.append({
            "xT": xT,
            "wq": np.ascontiguousarray(wq_f[:, sl], dtype=np.float16),
            "wk": np.ascontiguousarray(wk_f[:, sl], dtype=np.float16),
            "wv": np.ascontiguousarray(wv_f[:, sl], dtype=np.float16),
            "bq": np.ascontiguousarray(bq_f[sl, None], dtype=np.float32),
            "bk": np.ascontiguousarray(bk_f[sl, None], dtype=np.float32),
            "wfc": np.ascontiguousarray(w_fc64[sl, :], dtype=np.float16),
        })
    return in_maps, b_eff


def kernel(x, w_qkv, b_qkv, w_fc, b_fc, _trace=False):
    in_maps, b_eff = _prep_inputs(x, w_qkv, b_qkv, w_fc, b_fc)
    if "nc" not in _CACHE:
        _CACHE["nc"] = build_kernel()
    nc = _CACHE["nc"]
    res = run_bass_kernel_spmd(nc, in_maps, core_ids=list(range(N_CORES)),
                               trace=_trace)
    _CACHE["last_result"] = res
    acc = res.results[0]["outT"].astype(np.float64)
    for m in range(1, N_CORES):
        acc += res.results[m]["outT"]
    out = acc.T + b_eff[None, :]
    return np.ascontiguousarray(out.reshape(B, T, C).astype(np.float32))
